# revision 1
# baseline (speedup 1.0000x reference)
"""Trainium2 Bass kernel for nn_CollisonToJointLoss.

Math restructure (avoids the 3K*J*J elementwise blowup):
  joint_regressor >= 0, so where both gathered scores are nonzero,
  |intr_s + recv_s| = intr_s + recv_s.  For one collision-vertex pair
  (u = intruder vertex, w = receiver vertex, batch b):

    sum_{i,j} mask*d*(s_u[i]+s_w[j]) = s_u^T D_b m_w + m_u^T D_b s_w
    sum_{i,j} mask                   = m_u^T (11^T) m_w

  with s_v = jr[v], m_v = (s_v > 0), D_b = pairwise joint distances.
  Summing over pairs becomes one accumulated PE matmul per batch with
  rows [S|M]: ACC = [S|M]_intr^T @ [S|M]_recv, where
    TR = S_i^T M_r, BL = M_i^T S_r (the two num terms), BR = M_i^T M_r (den).
  num_b = <D_b, TR + BL>, den_b = sum(BR).

Gathers run via gpsimd.dma_gather (256B rows, int16 wrapped indices).
The wrapped index layout (idx r at partition r%16 replicated across
16-partition groups, slot r//16) is built with one contiguous
SBUF->DRAM write plus 8 per-group loads; slot assignments are chosen
so each collision's intruder/receiver rows land partition-aligned for
the matmul (side distance in the slot dimension divisible by 8).

Invalid collisions (cf[:,0] < 0) redirect both face ids to a padded
zero-face whose vertices point at an all-zero jr row => S = M = 0.

Sharding: data-parallel over batch B: 8 cores x 2 batches.  Each core
returns partial (num, den); host sums and finishes the mean.
"""

import numpy as np

B, C, N, F, J = 16, 2048, 6890, 13776, 24
NCORES = 8
BPC = B // NCORES          # batches per core
NPAD = 6912                # 128 * 54  (jr/verts padded with zero rows)
KCH = NPAD // 128          # 54 chunks for the joints matmul
CPP = C // 128             # 16 collisions per partition per batch
FPAD = F + 1               # faces rows per batch incl. zero-face row
J2 = 2 * J                 # 48
E = 64                     # gather row width (256B granularity)

_CACHE = {}


def _build_program():
    import concourse.bass as bass
    import concourse.tile as tile
    from concourse import bacc, mybir
    from concourse.masks import make_identity

    f32 = mybir.dt.float32
    i32 = mybir.dt.int32
    i16 = mybir.dt.int16
    Alu = mybir.AluOpType

    nc = bacc.Bacc("TRN2", target_bir_lowering=False, debug=False,
                   num_swdge_queues=2)

    jrt = nc.dram_tensor("jrt", [NPAD, J], f32, kind="ExternalInput").ap()
    verts = nc.dram_tensor("verts", [BPC, NPAD, 3], f32, kind="ExternalInput").ap()
    cf = nc.dram_tensor("cf", [BPC, C, 2], i32, kind="ExternalInput").ap()
    faces64 = nc.dram_tensor("faces64", [BPC * FPAD, E], i32,
                             kind="ExternalInput").ap()
    out_d = nc.dram_tensor("out", [2, 1], f32, kind="ExternalOutput").ap()

    with tile.TileContext(nc) as tc:
        with tc.tile_pool(name="sb", bufs=1) as sb, \
             tc.tile_pool(name="pp", bufs=1, space="PSUM") as pp, \
             tc.tile_pool(name="dp", bufs=1, space="DRAM") as dp:

            # ---- bulk loads -------------------------------------------------
            JT = sb.tile([128, KCH, J], f32)        # jr rows 54p+k at (p, k)
            nc.sync.dma_start(out=JT[:].rearrange("p k j -> p (k j)"),
                              in_=jrt.rearrange("(p k) j -> p (k j)", p=128))

            VT0 = sb.tile([128, KCH, 3], f32)
            VT1 = sb.tile([128, KCH, 3], f32)
            nc.sync.dma_start(out=VT0[:].rearrange("p k d -> p (k d)"),
                              in_=verts[0].rearrange("(p k) d -> p (k d)", p=128))
            nc.sync.dma_start(out=VT1[:].rearrange("p k d -> p (k d)"),
                              in_=verts[1].rearrange("(p k) d -> p (k d)", p=128))
            VC = sb.tile([128, KCH, 6], f32)
            nc.vector.tensor_copy(out=VC[:, :, 0:3], in_=VT0[:])
            nc.vector.tensor_copy(out=VC[:, :, 3:6], in_=VT1[:])

            CFt = sb.tile([128, BPC * CPP * 2], i32)
            for b in range(BPC):
                nc.sync.dma_start(
                    out=CFt[:, b * CPP * 2:(b + 1) * CPP * 2],
                    in_=cf[b].rearrange("(p n) s -> p (n s)", p=128))

            ident = sb.tile([128, 128], f32)
            make_identity(nc, ident[:])

            # ---- [S | M | pad] gather table in DRAM -------------------------
            TSM = sb.tile([128, KCH, E], f32)
            nc.vector.tensor_copy(out=TSM[:, :, 0:J], in_=JT[:])
            nc.vector.tensor_scalar(out=TSM[:, :, J:J2], in0=JT[:],
                                    scalar1=0.0, scalar2=None, op0=Alu.is_gt)
            nc.vector.memset(TSM[:, :, J2:E], 0.0)
            tsm = dp.tile([NPAD, E], f32)
            nc.sync.dma_start(out=tsm[:].rearrange("(p k) j -> p (k j)", p=128),
                              in_=TSM[:].rearrange("p k j -> p (k j)"))

            # ---- joints = jr^T-chunks contracted with verts ------------------
            J6p = pp.tile([J, 6], f32)
            for k in range(KCH):
                nc.tensor.matmul(out=J6p[:], lhsT=JT[:, k, :], rhs=VC[:, k, :],
                                 start=(k == 0), stop=(k == KCH - 1))
            j6 = sb.tile([J, 6], f32)
            nc.vector.tensor_copy(out=j6[:], in_=J6p[:])
            j6sq = sb.tile([J, 6], f32)
            nc.vector.tensor_mul(out=j6sq[:], in0=j6[:], in1=j6[:])

            ones3_24 = sb.tile([3, J], f32)
            nc.vector.memset(ones3_24[:], 1.0)
            ones24 = sb.tile([J, 1], f32)
            nc.vector.memset(ones24[:], 1.0)

            # jt_b = joints_b^T [3, 24] at partition base 0
            jtp = pp.tile([3, J2], f32)
            for b in range(BPC):
                nc.tensor.transpose(out=jtp[:, J * b:J * b + J],
                                    in_=j6[:, 3 * b:3 * b + 3],
                                    identity=ident[:J, :J])
            jt = sb.tile([3, J2], f32)
            nc.vector.tensor_copy(out=jt[:], in_=jtp[:])
            sq = sb.tile([3, J2], f32)
            nc.vector.tensor_mul(out=sq[:], in0=jt[:], in1=jt[:])
            jtm2 = sb.tile([3, J2], f32)
            nc.vector.tensor_scalar_mul(out=jtm2[:], in0=jt[:], scalar1=-2.0)

            # ---- D_b = pairwise joint distances (partitions 0-23) -----------
            G24 = pp.tile([J, J], f32)
            D_b = [sb.tile([J, J], f32, name=f"D{b}") for b in range(BPC)]
            rc = sb.tile([J, BPC], f32)
            for b in range(BPC):
                nc.vector.reduce_sum(out=rc[:, b:b + 1], in_=j6sq[:, 3 * b:3 * b + 3],
                                     axis=mybir.AxisListType.X)
            for b in range(BPC):
                nc.tensor.matmul(out=G24[:], lhsT=jtm2[:, J * b:J * b + J],
                                 rhs=jt[:, J * b:J * b + J], start=True, stop=False)
                nc.tensor.matmul(out=G24[:], lhsT=ones3_24[:],
                                 rhs=sq[:, J * b:J * b + J], start=False, stop=True)
                nc.vector.tensor_scalar(out=D_b[b][:], in0=G24[:],
                                        scalar1=rc[:, b:b + 1], scalar2=0.0,
                                        op0=Alu.add, op1=Alu.max)
                nc.scalar.activation(out=D_b[b][:], in_=D_b[b][:],
                                     func=mybir.ActivationFunctionType.Sqrt)

            # ---- collision index processing (both batches) ------------------
            CFv = CFt[:].rearrange("p (b n s) -> p b n s", b=BPC, s=2)
            CFf = sb.tile([128, BPC, CPP, 2], f32)
            nc.vector.tensor_copy(out=CFf[:], in_=CFv)
            V0 = sb.tile([128, BPC, CPP, 1], f32)
            nc.vector.tensor_scalar(out=V0[:], in0=CFf[:, :, :, 0:1],
                                    scalar1=0.0, scalar2=None, op0=Alu.is_ge)
            nc.vector.tensor_scalar_max(out=CFf[:], in0=CFf[:], scalar1=0.0)
            # sel = valid ? cf : F  (zero-face row in faces64_b)
            T1 = sb.tile([128, BPC, CPP, 2], f32)
            nc.vector.tensor_tensor(out=T1[:], in0=CFf[:],
                                    in1=V0[:].to_broadcast([128, BPC, CPP, 2]),
                                    op=Alu.mult)
            T2 = sb.tile([128, BPC, CPP, 1], f32)
            nc.vector.tensor_scalar(out=T2[:], in0=V0[:],
                                    scalar1=-float(F), scalar2=float(F),
                                    op0=Alu.mult, op1=Alu.add)
            nc.vector.tensor_tensor(out=T1[:], in0=T1[:],
                                    in1=T2[:].to_broadcast([128, BPC, CPP, 2]),
                                    op=Alu.add)
            for b in range(1, BPC):
                nc.vector.tensor_scalar_add(out=T1[:, b], in0=T1[:, b],
                                            scalar1=float(b * FPAD))

            # ---- merged gather pipeline (both batches in one shot) ----------
            NFI = BPC * CPP * 2 * 128      # 8192 face indices
            NJI = BPC * CPP * 2 * 3 * 128  # 24576 jr indices
            ACC = [pp.tile([J2, J2], f32, name=f"ACC{b}") for b in range(BPC)]
            # face ids int16, slot cF = 32b + 16s + n
            CFb16 = sb.tile([128, NFI // 128], i16)
            nc.vector.tensor_copy(
                out=CFb16[:].rearrange("p (b s n) -> p b s n", b=BPC, s=2),
                in_=T1[:].rearrange("p b n s -> p b s n"))
            # wrap-write: dramF[q16, phi, cF] = CFb16[16phi+q16, cF]
            dramF = dp.tile([16, 8, NFI // 128], i16)
            nc.sync.dma_start(out=dramF[:].rearrange("q phi c -> phi q c"),
                              in_=CFb16[:])
            WF = sb.tile([128, NFI // 16], i16)
            for g in range(8):
                nc.sync.dma_start(out=WF[16 * g:16 * (g + 1), :],
                                  in_=dramF[:].rearrange("q phi c -> q (phi c)"))
            # face gather: r=(64phi+cF)*16+q16 -> VB[16(cF%8)+q16, 8phi+cF//8]
            VB = sb.tile([128, NFI // 128, E], i32)
            nc.gpsimd.dma_gather(out_ap=VB[:], in_ap=faces64,
                                 idxs_ap=WF[:], num_idxs=NFI,
                                 num_idxs_reg=NFI, elem_size=E,
                                 single_packet=False)
            # vertex ids -> int16, mm = 96b + 48s + 6phi + 3kap + t
            # (c_vb = 8phi + 4b + 2s + kap; split by s to stay <=4 AP dims)
            VB16 = sb.tile([128, NJI // 128], i16)
            for s in range(2):
                vbv = VB[:, :, 0:3].rearrange("p (phi b s k) t -> p s b phi k t",
                                              phi=8, b=BPC, s=2)
                nc.vector.tensor_copy(
                    out=VB16[:].rearrange("p (b s phi k t) -> p s b phi k t",
                                          b=BPC, s=2, phi=8, k=2)[:, s],
                    in_=vbv[:, s])
            # wrap-write per batch: dramJ[b, q16, phi2, mmb]
            #   = VB16[16phi2+q16, 96b+mmb]
            NJB = NJI // BPC               # 12288 jr indices per batch
            dramJ = dp.tile([BPC, 16, 8, 96], i16)
            for b in range(BPC):
                nc.sync.dma_start(
                    out=dramJ[b].rearrange("q phi m -> phi q m"),
                    in_=VB16[:, b * 96:(b + 1) * 96])
            WJ = sb.tile([128, NJI // 16], i16)
            for g in range(8):
                nc.sync.dma_start(
                    out=WJ[16 * g:16 * (g + 1), :],
                    in_=dramJ[:].rearrange("b q phi m -> q b (phi m)"))
            # jr gather per batch (SWDGE ring caps ~1024 descriptors):
            # local r2=(96phi2+mmb)*16+q16 -> U[16(mmb%8)+q16, 96b+12phi2+mmb//8]
            U = sb.tile([128, NJI // 128, E], f32)
            for b in range(BPC):
                nc.gpsimd.dma_gather(
                    out_ap=U[:, b * 96:(b + 1) * 96, :], in_ap=tsm[:],
                    idxs_ap=WJ[:, b * (NJB // 16):(b + 1) * (NJB // 16)],
                    num_idxs=NJB, num_idxs_reg=NJB, elem_size=E,
                    single_packet=False, queue_num=b)

            # ---- main accumulation ------------------------------------------
            for b in range(BPC):
                first = True
                for nu in range(8):
                    for mu in range(6):
                        c = 96 * b + 12 * nu + mu
                        nc.tensor.matmul(out=ACC[b][:],
                                         lhsT=U[:, c + 6, 0:J2],
                                         rhs=U[:, c, 0:J2],
                                         start=first,
                                         stop=(nu == 7 and mu == 5))
                        first = False

            # ---- final reductions -------------------------------------------
            # BRBL = ACC[24:48, :] moved to partitions 0-23 via selector matmul
            OUT2 = pp.tile([2, 1], f32)
            BRBL = pp.tile([J, J2], f32)
            Vc = [sb.tile([J, 2], f32, name=f"Vc{b}") for b in range(BPC)]
            for b in range(BPC):
                ACCs = sb.tile([J2, J2], f32, name=f"ACCs{b}")
                nc.vector.tensor_copy(out=ACCs[:], in_=ACC[b][:])
                nc.tensor.matmul(out=BRBL[:], lhsT=ident[0:J2, J:J2],
                                 rhs=ACCs[:], start=True, stop=True)
                # num terms: TR = ACCs[0:24, 24:48], BL = BRBL[:, 0:24]
                NU = sb.tile([J, J2], f32, name=f"NU{b}")
                nc.vector.tensor_mul(out=NU[:, 0:J], in0=ACCs[0:J, J:J2],
                                     in1=D_b[b][:])
                nc.vector.tensor_mul(out=NU[:, J:J2], in0=BRBL[:, 0:J],
                                     in1=D_b[b][:])
                nc.vector.reduce_sum(out=Vc[b][:, 0:1], in_=NU[:],
                                     axis=mybir.AxisListType.X)
                nc.vector.reduce_sum(out=Vc[b][:, 1:2], in_=BRBL[:, J:J2],
                                     axis=mybir.AxisListType.X)
            for b in range(BPC):
                nc.tensor.matmul(out=OUT2[:], lhsT=Vc[b][:], rhs=ones24[:],
                                 start=(b == 0), stop=(b == BPC - 1))

            outs = sb.tile([2, 1], f32)
            nc.vector.tensor_copy(out=outs[:], in_=OUT2[:])
            nc.sync.dma_start(out=out_d, in_=outs[:])

    nc.compile()
    return nc


def get_program():
    if "nc" not in _CACHE:
        _CACHE["nc"] = _build_program()
    return _CACHE["nc"]


def make_in_maps(collision_idxs, vertices, faces, joint_regressor):
    """Host-side shard/layout prep. Returns list of per-core input dicts."""
    collision_idxs = np.asarray(collision_idxs)
    vertices = np.asarray(vertices)
    faces = np.asarray(faces)
    joint_regressor = np.asarray(joint_regressor)
    jrt = np.zeros((NPAD, J), dtype=np.float32)
    jrt[:N, :] = np.ascontiguousarray(joint_regressor.T.astype(np.float32))

    vpad = np.zeros((B, NPAD, 3), dtype=np.float32)
    vpad[:, :N, :] = vertices.astype(np.float32)

    cfi = collision_idxs.astype(np.int32)
    # faces rows embedded in 256B rows; final row = zero-face -> zero jr row
    f64 = np.zeros((B, FPAD, E), dtype=np.int32)
    f64[:, :F, 0:3] = faces.astype(np.int32)
    f64[:, F, 0:3] = N

    in_maps = []
    for c in range(NCORES):
        bs = slice(c * BPC, (c + 1) * BPC)
        m = {
            "jrt": jrt,
            "verts": np.ascontiguousarray(vpad[bs]),
            "cf": np.ascontiguousarray(cfi[bs]),
        }
        m["faces64"] = np.ascontiguousarray(
            f64[bs].reshape(BPC * FPAD, E))
        in_maps.append(m)
    return in_maps


def kernel(collision_idxs, vertices, faces, joint_regressor):
    from concourse.bass_utils import run_bass_kernel_spmd

    nc = get_program()
    in_maps = make_in_maps(collision_idxs, vertices, faces, joint_regressor)
    res = run_bass_kernel_spmd(nc, in_maps, core_ids=list(range(NCORES)))
    num = 0.0
    den = 0.0
    for r in res.results:
        o = np.asarray(r["out"], dtype=np.float64).reshape(-1)
        num += o[0]
        den += o[1]
    if den > 0:
        val = num / max(den, 1.0)
    else:
        val = 0.0
    return np.float32(val)



# revision 2
# speedup vs baseline: 4.5230x; 4.5230x over previous
"""Trainium2 Bass kernel for nn_CollisonToJointLoss.

Math restructure (same identity as the earlier version): jr >= 0, so where
both gathered scores are nonzero, |intr_s + recv_s| = intr_s + recv_s, and

    num_b = <D_b, Sum_c Sum_t [S|M]_intr^T [S|M]_recv  (TR + BL blocks)>
    den_b = sum(BR block),    with S_v = jr[v], M_v = (S_v > 0).

Key layout change vs the 104us version: the gather table is indexed BY FACE,
not by vertex.  Each 512B table row holds the face's three [S|M] vertex rows
([3 x 48] bf16 = 288B used).  One collision side therefore costs ONE gather
descriptor instead of 1 (face->verts) + 3 (vert->jr) descriptors of 256B
each: 8192 descriptors total instead of 32768, i.e. ~11.7us of DMA instead
of ~46.6us under the 22.76ns/descriptor DMA cost (256B and 512B descriptors
cost the same; the floor is at work per descriptor, not bytes).

The gather index array is host-prepared in the HW wrapped layout
([16, n/16] int16, replicated across the eight 16-partition groups), with
invalid collisions (cf[:,0] < 0) redirected to an all-zero table row and the
per-batch table base (+b*FPAD) folded in.  The gather runs as 4 chunks of
2048 descriptors so PE accumulation overlaps later chunks, and batch 0's
final reduction overlaps batch 1's gather.

Sharding: data-parallel over batch B: 8 cores x 2 batches.  Each core
returns partial (num, den); host sums and finishes the mean.
"""

import numpy as np

B, C, N, F, J = 16, 2048, 6890, 13776, 24
NCORES = 8
BPC = B // NCORES          # batches per core
NPAD = 6912                # 128 * 54  (jr/verts padded with zero rows)
KCH = NPAD // 128          # 54 chunks for the joints matmul
FPAD = F + 1               # table rows per batch incl. zero-face row
J2 = 2 * J                 # 48
E = 256                    # table row width in bf16 elems (512B stride)
NIDX = BPC * C * 2         # 8192 gather descriptors per core
NCHUNK = 4
CIDX = NIDX // NCHUNK      # 2048 descriptors per gather chunk

_CACHE = {}


def _build_program():
    import concourse.bass as bass
    import concourse.tile as tile
    from concourse import bacc, mybir
    from concourse.masks import make_identity

    f32 = mybir.dt.float32
    bf16 = mybir.dt.bfloat16
    i16 = mybir.dt.int16
    Alu = mybir.AluOpType

    nc = bacc.Bacc("TRN2", target_bir_lowering=False, debug=False,
                   num_swdge_queues=2)

    widx_d = nc.dram_tensor("widx", [128, NIDX // 16], i16,
                            kind="ExternalInput").ap()
    jrt_d = nc.dram_tensor("jrt", [128, KCH * J], f32,
                           kind="ExternalInput").ap()
    vc_d = nc.dram_tensor("vc", [128, KCH * 6], f32,
                          kind="ExternalInput").ap()
    fsm_d = nc.dram_tensor("fsm", [BPC * FPAD, E], bf16,
                           kind="ExternalInput").ap()
    out_d = nc.dram_tensor("out", [2, 1], f32, kind="ExternalOutput").ap()

    with tile.TileContext(nc) as tc:
        with tc.tile_pool(name="sb", bufs=1) as sb, \
             tc.tile_pool(name="pp", bufs=1, space="PSUM") as pp:

            # ---- bulk loads (widx first: it gates the gather pipeline) ----
            WIDX = sb.tile([128, NIDX // 16], i16)
            nc.sync.dma_start(out=WIDX[:], in_=widx_d)
            JT = sb.tile([128, KCH, J], f32)
            nc.sync.dma_start(out=JT[:].rearrange("p k j -> p (k j)"),
                              in_=jrt_d)
            VC = sb.tile([128, KCH, 6], f32)
            nc.sync.dma_start(out=VC[:].rearrange("p k d -> p (k d)"),
                              in_=vc_d)

            ident = sb.tile([128, 128], f32)
            make_identity(nc, ident[:])
            ones3_24 = sb.tile([3, J], f32)
            nc.vector.memset(ones3_24[:], 1.0)
            ones24 = sb.tile([J, 1], f32)
            nc.vector.memset(ones24[:], 1.0)

            # ---- joints = jr^T-chunks contracted with verts ----------------
            J6p = pp.tile([J, 6], f32)
            for k in range(KCH):
                nc.tensor.matmul(out=J6p[:], lhsT=JT[:, k, :], rhs=VC[:, k, :],
                                 start=(k == 0), stop=(k == KCH - 1))
            j6 = sb.tile([J, 6], f32)
            nc.vector.tensor_copy(out=j6[:], in_=J6p[:])
            j6sq = sb.tile([J, 6], f32)
            nc.vector.tensor_mul(out=j6sq[:], in0=j6[:], in1=j6[:])

            # jt_b = joints_b^T [3, 24] at partition base 0
            jtp = pp.tile([3, J2], f32)
            for b in range(BPC):
                nc.tensor.transpose(out=jtp[:, J * b:J * b + J],
                                    in_=j6[:, 3 * b:3 * b + 3],
                                    identity=ident[:J, :J])
            jt = sb.tile([3, J2], f32)
            nc.vector.tensor_copy(out=jt[:], in_=jtp[:])
            sq = sb.tile([3, J2], f32)
            nc.vector.tensor_mul(out=sq[:], in0=jt[:], in1=jt[:])
            jtm2 = sb.tile([3, J2], f32)
            nc.vector.tensor_scalar_mul(out=jtm2[:], in0=jt[:], scalar1=-2.0)

            # ---- D_b = pairwise joint distances (partitions 0-23) ---------
            G24 = pp.tile([J, J], f32)
            D_b = [sb.tile([J, J], f32, name=f"D{b}") for b in range(BPC)]
            rc = sb.tile([J, BPC], f32)
            for b in range(BPC):
                nc.vector.reduce_sum(out=rc[:, b:b + 1],
                                     in_=j6sq[:, 3 * b:3 * b + 3],
                                     axis=mybir.AxisListType.X)
            for b in range(BPC):
                nc.tensor.matmul(out=G24[:], lhsT=jtm2[:, J * b:J * b + J],
                                 rhs=jt[:, J * b:J * b + J], start=True,
                                 stop=False)
                nc.tensor.matmul(out=G24[:], lhsT=ones3_24[:],
                                 rhs=sq[:, J * b:J * b + J], start=False,
                                 stop=True)
                nc.vector.tensor_scalar(out=D_b[b][:], in0=G24[:],
                                        scalar1=rc[:, b:b + 1], scalar2=0.0,
                                        op0=Alu.add, op1=Alu.max)
                nc.scalar.activation(out=D_b[b][:], in_=D_b[b][:],
                                     func=mybir.ActivationFunctionType.Sqrt)

            # ---- chunked face-row gathers ---------------------------------
            # descriptor k = T*256 + side*128 + p; table row holds the face's
            # 3 [S|M] vertex rows at 48-elem offsets.  U slot = 2T + side.
            U = sb.tile([128, NIDX // 128, E], bf16)
            for ch in range(NCHUNK):
                ns = CIDX // 128       # slots per chunk (16)
                nc.gpsimd.dma_gather(
                    out_ap=U[:, ch * ns:(ch + 1) * ns, :], in_ap=fsm_d,
                    idxs_ap=WIDX[:, ch * (CIDX // 16):(ch + 1) * (CIDX // 16)],
                    num_idxs=CIDX, num_idxs_reg=CIDX, elem_size=E,
                    single_packet=False, queue_num=ch % 2)

            # ---- accumulate ACC_b = Sum [S|M]_intr^T [S|M]_recv -----------
            # chunks 0-1 cover batch 0 (tiles T 0-15), chunks 2-3 batch 1.
            ACC = [pp.tile([J2, J2], f32, name=f"ACC{b}") for b in range(BPC)]
            BRBL = [pp.tile([J, J2], f32, name=f"BRBL{b}") for b in range(BPC)]
            Vc = [sb.tile([J, 2], f32, name=f"Vc{b}") for b in range(BPC)]
            OUT2 = pp.tile([2, 1], f32)

            def acc_matmuls(b):
                first = True
                for T in range(16 * b, 16 * (b + 1)):
                    for tau in range(3):
                        nc.tensor.matmul(
                            out=ACC[b][:],
                            lhsT=U[:, 2 * T + 1, J2 * tau:J2 * (tau + 1)],
                            rhs=U[:, 2 * T, J2 * tau:J2 * (tau + 1)],
                            start=first, stop=(T == 16 * b + 15 and tau == 2))
                        first = False

            def reduction(b):
                ACCs = sb.tile([J2, J2], f32, name=f"ACCs{b}")
                nc.vector.tensor_copy(out=ACCs[:], in_=ACC[b][:])
                nc.tensor.matmul(out=BRBL[b][:], lhsT=ident[0:J2, J:J2],
                                 rhs=ACCs[:], start=True, stop=True)
                NU = sb.tile([J, J2], f32, name=f"NU{b}")
                nc.vector.tensor_mul(out=NU[:, 0:J], in0=ACCs[0:J, J:J2],
                                     in1=D_b[b][:])
                nc.vector.tensor_mul(out=NU[:, J:J2], in0=BRBL[b][:, 0:J],
                                     in1=D_b[b][:])
                nc.vector.reduce_sum(out=Vc[b][:, 0:1], in_=NU[:],
                                     axis=mybir.AxisListType.X)
                nc.vector.reduce_sum(out=Vc[b][:, 1:2], in_=BRBL[b][:, J:J2],
                                     axis=mybir.AxisListType.X)

            acc_matmuls(0)
            reduction(0)
            acc_matmuls(1)
            reduction(1)

            for b in range(BPC):
                nc.tensor.matmul(out=OUT2[:], lhsT=Vc[b][:], rhs=ones24[:],
                                 start=(b == 0), stop=(b == BPC - 1))
            outs = sb.tile([2, 1], f32)
            nc.vector.tensor_copy(out=outs[:], in_=OUT2[:])
            nc.sync.dma_start(out=out_d, in_=outs[:])

    nc.compile()
    return nc


def get_program():
    if "nc" not in _CACHE:
        _CACHE["nc"] = _build_program()
    return _CACHE["nc"]


def make_in_maps(collision_idxs, vertices, faces, joint_regressor):
    """Host-side shard/layout prep. Returns list of per-core input dicts."""
    import ml_dtypes
    bf16 = ml_dtypes.bfloat16

    collision_idxs = np.asarray(collision_idxs)
    vertices = np.asarray(vertices)
    faces = np.asarray(faces).astype(np.int64)
    joint_regressor = np.asarray(joint_regressor)

    # jr^T padded, f32 for the joints matmul; [S|M] rows in bf16 for the table
    jrt = np.zeros((NPAD, J), dtype=np.float32)
    jrt[:N, :] = joint_regressor.T.astype(np.float32)
    sm = np.zeros((NPAD, J2), dtype=bf16)
    sm[:N, 0:J] = jrt[:N].astype(bf16)
    sm[:N, J:J2] = (jrt[:N] != 0).astype(bf16)

    # per-(batch, face) table row: 3 x [S|M] = 144 bf16, padded to 256
    fsm_all = np.zeros((B, FPAD, E), dtype=bf16)
    fsm_all[:, :F, 0:3 * J2] = sm[faces.reshape(B, F * 3)].reshape(B, F, 3 * J2)

    vpad = np.zeros((B, NPAD, 3), dtype=np.float32)
    vpad[:, :N, :] = vertices.astype(np.float32)

    # gather index values: valid ? clip(cf) : F (zero row), + b*FPAD
    cidx = collision_idxs.astype(np.int32)
    valid = cidx[:, :, 0] >= 0
    sel = np.empty((2, B, C), dtype=np.int32)      # side 0 = recv, 1 = intr
    sel[0] = np.where(valid, np.maximum(cidx[:, :, 0], 0), F)
    sel[1] = np.where(valid, np.maximum(cidx[:, :, 1], 0), F)

    # c(q, t, a) = q*128 + t*8 + a; descriptor k = T*256 + side*128 + 16a + q
    cgrid = (np.arange(16)[:, None, None] * 128 +
             np.arange(16)[None, :, None] * 8 +
             np.arange(8)[None, None, :])          # [q, t, a]

    in_maps = []
    for core in range(NCORES):
        bs = slice(core * BPC, (core + 1) * BPC)
        v = np.empty((2 * 16, 2, 8, 16), dtype=np.int32)   # [T, side, a, q]
        for bb in range(BPC):
            for side in range(2):
                g = sel[side, core * BPC + bb][cgrid]      # [q, t, a]
                v[bb * 16:(bb + 1) * 16, side] = (
                    bb * FPAD + g.transpose(1, 2, 0))      # [t, a, q]
        wrapped = v.reshape(NIDX // 16, 16).T              # [q, slot]
        widx = np.tile(wrapped, (8, 1)).astype(np.int16)

        vc = np.zeros((NPAD, 6), dtype=np.float32)
        vc[:, 0:3] = vpad[core * BPC]
        vc[:, 3:6] = vpad[core * BPC + 1]

        in_maps.append({
            "widx": widx,
            "jrt": np.ascontiguousarray(jrt.reshape(128, KCH * J)),
            "vc": np.ascontiguousarray(vc.reshape(128, KCH * 6)),
            "fsm": np.ascontiguousarray(
                fsm_all[bs].reshape(BPC * FPAD, E)),
        })
    return in_maps


def kernel(collision_idxs, vertices, faces, joint_regressor):
    from concourse.bass_utils import run_bass_kernel_spmd

    nc = get_program()
    in_maps = make_in_maps(collision_idxs, vertices, faces, joint_regressor)
    res = run_bass_kernel_spmd(nc, in_maps, core_ids=list(range(NCORES)))
    num = 0.0
    den = 0.0
    for r in res.results:
        o = np.asarray(r["out"], dtype=np.float64).reshape(-1)
        num += o[0]
        den += o[1]
    if den > 0:
        val = num / max(den, 1.0)
    else:
        val = 0.0
    return np.float32(val)


# revision 11
# speedup vs baseline: 4.7598x; 1.0524x over previous
"""Trainium2 Bass kernel for nn_CollisonToJointLoss.

Math restructure (same identity as the earlier version): jr >= 0, so where
both gathered scores are nonzero, |intr_s + recv_s| = intr_s + recv_s, and

    num_b = <D_b, Sum_c Sum_t [S|M]_intr^T [S|M]_recv  (TR + BL blocks)>
    den_b = sum(BR block),    with S_v = jr[v], M_v = (S_v > 0).

Key layout change vs the 104us version: the gather table is indexed BY FACE,
not by vertex.  Each 512B table row holds the face's three [S|M] vertex rows
([3 x 48] bf16 = 288B used).  One collision side therefore costs ONE gather
descriptor instead of 1 (face->verts) + 3 (vert->jr) descriptors of 256B
each: 8192 descriptors total instead of 32768, i.e. ~11.7us of DMA instead
of ~46.6us under the 22.76ns/descriptor DMA cost (256B and 512B descriptors
cost the same; the floor is at work per descriptor, not bytes).

The gather index array is host-prepared in the HW wrapped layout
([16, n/16] int16, replicated across the eight 16-partition groups), with
invalid collisions (cf[:,0] < 0) redirected to an all-zero table row and the
per-batch table base (+b*FPAD) folded in.  The gather runs as 4 chunks of
2048 descriptors so PE accumulation overlaps later chunks, and batch 0's
final reduction overlaps batch 1's gather.

Sharding: data-parallel over batch B: 8 cores x 2 batches.  Each core
returns partial (num, den); host sums and finishes the mean.
"""

import numpy as np

B, C, N, F, J = 16, 2048, 6890, 13776, 24
NCORES = 8
BPC = B // NCORES          # batches per core
NPAD = 6912                # 128 * 54  (jr/verts padded with zero rows)
KCH = NPAD // 128          # 54 chunks for the joints matmul
FPAD = F + 1               # table rows per batch incl. zero-face row
J2 = 2 * J                 # 48
E = 256                    # table row width in bf16 elems (512B stride)
NIDX = BPC * C * 2         # 8192 gather descriptors per core
NCHUNK = 4
CIDX = NIDX // NCHUNK      # 2048 descriptors per gather chunk

_CACHE = {}


def _build_program():
    import concourse.bass as bass
    import concourse.tile as tile
    from concourse import bacc, mybir
    from concourse.masks import make_identity

    f32 = mybir.dt.float32
    bf16 = mybir.dt.bfloat16
    i16 = mybir.dt.int16
    Alu = mybir.AluOpType

    nc = bacc.Bacc("TRN2", target_bir_lowering=False, debug=False,
                   num_swdge_queues=2)

    widx_d = nc.dram_tensor("widx", [128, NIDX // 16], i16,
                            kind="ExternalInput").ap()
    jrt_d = nc.dram_tensor("jrt", [128, KCH * J], f32,
                           kind="ExternalInput").ap()
    vc_d = nc.dram_tensor("vc", [128, KCH * 6], f32,
                          kind="ExternalInput").ap()
    fsm_d = nc.dram_tensor("fsm", [BPC * FPAD, E], bf16,
                           kind="ExternalInput").ap()
    msk_d = nc.dram_tensor("msk", [J2, J2], f32, kind="ExternalInput").ap()
    out_d = nc.dram_tensor("out", [J2, 4], f32, kind="ExternalOutput").ap()

    with tile.TileContext(nc) as tc:
        with tc.tile_pool(name="sb", bufs=1) as sb, \
             tc.tile_pool(name="pp", bufs=1, space="PSUM") as pp:

            # ---- bulk loads (widx first: it gates the gather pipeline) ----
            WIDX = sb.tile([128, NIDX // 16], i16)
            nc.sync.dma_start(out=WIDX[:], in_=widx_d)
            JT = sb.tile([128, KCH, J], f32)
            nc.sync.dma_start(out=JT[:].rearrange("p k j -> p (k j)"),
                              in_=jrt_d)
            VC = sb.tile([128, KCH, 6], f32)
            nc.sync.dma_start(out=VC[:].rearrange("p k d -> p (k d)"),
                              in_=vc_d)

            MSK = sb.tile([J2, J2], f32)
            nc.sync.dma_start(out=MSK[:], in_=msk_d)
            ident = sb.tile([128, 128], f32)
            make_identity(nc, ident[:])

            # ---- joints = jr^T-chunks contracted with verts ----------------
            J6p = pp.tile([J, 6], f32)
            for k in range(KCH):
                nc.tensor.matmul(out=J6p[:], lhsT=JT[:, k, :], rhs=VC[:, k, :],
                                 start=(k == 0), stop=(k == KCH - 1))
            j6 = sb.tile([J, 6], f32)
            nc.vector.tensor_copy(out=j6[:], in_=J6p[:])

            # jt_b^T [3, 24] duplicated to [3, 48] so the squared-distance
            # matrix lands on all 48 partitions directly (D48 blocks = D).
            jtp = pp.tile([3, J2], f32)
            for b in range(BPC):
                nc.tensor.transpose(out=jtp[:, J * b:J * b + J],
                                    in_=j6[:, 3 * b:3 * b + 3],
                                    identity=ident[:J, :J])
            jtd = [sb.tile([3, J2], f32, name=f"jtd{b}") for b in range(BPC)]
            sqd = [sb.tile([3, J2], f32, name=f"sqd{b}") for b in range(BPC)]
            jtm2 = [sb.tile([3, J2], f32, name=f"jtm2{b}") for b in range(BPC)]
            ones3_48 = sb.tile([3, J2], f32)
            nc.vector.memset(ones3_48[:], 1.0)
            for b in range(BPC):
                nc.vector.tensor_copy(out=jtd[b][:, 0:J],
                                      in_=jtp[:, J * b:J * b + J])
                nc.vector.tensor_copy(out=jtd[b][:, J:J2],
                                      in_=jtp[:, J * b:J * b + J])
                nc.vector.tensor_mul(out=sqd[b][:], in0=jtd[b][:],
                                     in1=jtd[b][:])
                nc.vector.tensor_scalar_mul(out=jtm2[b][:], in0=jtd[b][:],
                                            scalar1=-2.0)

            # ---- DD_b: pairwise joint distances on 48 partitions, with the
            # diagonal blocks zeroed (only TR/BL of ACC contribute to num)
            G48 = pp.tile([J2, J2], f32)
            DD = [sb.tile([J2, J2], f32, name=f"DD{b}") for b in range(BPC)]
            for b in range(BPC):
                nc.tensor.matmul(out=G48[:], lhsT=jtm2[b][:], rhs=jtd[b][:],
                                 start=True, stop=False)
                nc.tensor.matmul(out=G48[:], lhsT=ones3_48[:], rhs=sqd[b][:],
                                 start=False, stop=False)
                nc.tensor.matmul(out=G48[:], lhsT=sqd[b][:], rhs=ones3_48[:],
                                 start=False, stop=True)
                nc.vector.tensor_scalar_max(out=DD[b][:], in0=G48[:],
                                            scalar1=0.0)
                nc.scalar.activation(out=DD[b][:], in_=DD[b][:],
                                     func=mybir.ActivationFunctionType.Sqrt)
                nc.vector.tensor_mul(out=DD[b][:], in0=DD[b][:], in1=MSK[:])

            # ---- chunked face-row gathers ---------------------------------
            # descriptor k = T*256 + side*128 + p; table row holds the face's
            # 3 [S|M] vertex rows at 48-elem offsets.  U slot = 2T + side.
            # Chunk sizes taper so the post-gather tail is short; desc-gen
            # (994 + 0.34/desc) stays ahead of the 1.42ns/desc transfers.
            U = sb.tile([128, NIDX // 128, E], bf16)
            bounds = [0, 2048, 4096, 6144, 7680, 8192]
            for ch in range(len(bounds) - 1):
                k0, k1 = bounds[ch], bounds[ch + 1]
                nc.gpsimd.dma_gather(
                    out_ap=U[:, k0 // 128:k1 // 128, :], in_ap=fsm_d,
                    idxs_ap=WIDX[:, k0 // 16:k1 // 16],
                    num_idxs=k1 - k0, num_idxs_reg=k1 - k0, elem_size=E,
                    single_packet=False, queue_num=ch % 2)

            # ---- accumulate ACC_b = Sum [S|M]_intr^T [S|M]_recv -----------
            # tiles T 0-15 are batch 0, 16-31 batch 1 (chunks in T order).
            ACC = [pp.tile([J2, J2], f32, name=f"ACC{b}") for b in range(BPC)]
            VV = sb.tile([J2, 4], f32)
            nc.vector.memset(VV[:], 0.0)

            def acc_matmuls(b):
                first = True
                for T in range(16 * b, 16 * (b + 1)):
                    for tau in range(3):
                        nc.tensor.matmul(
                            out=ACC[b][:],
                            lhsT=U[:, 2 * T + 1, J2 * tau:J2 * (tau + 1)],
                            rhs=U[:, 2 * T, J2 * tau:J2 * (tau + 1)],
                            start=first, stop=(T == 16 * b + 15 and tau == 2))
                        first = False

            def reduction(b):
                NU = sb.tile([J2, J2], f32, name=f"NU{b}")
                nc.vector.tensor_mul(out=NU[:], in0=ACC[b][:], in1=DD[b][:])
                nc.vector.reduce_sum(out=VV[:, 2 * b:2 * b + 1], in_=NU[:],
                                     axis=mybir.AxisListType.X)
                nc.vector.reduce_sum(out=VV[:, 2 * b + 1:2 * b + 2],
                                     in_=ACC[b][:, J:J2],
                                     axis=mybir.AxisListType.X)

            acc_matmuls(0)
            reduction(0)
            acc_matmuls(1)
            reduction(1)
            nc.sync.dma_start(out=out_d, in_=VV[:])

    nc.compile()
    return nc


def get_program():
    if "nc" not in _CACHE:
        _CACHE["nc"] = _build_program()
    return _CACHE["nc"]


def make_in_maps(collision_idxs, vertices, faces, joint_regressor):
    """Host-side shard/layout prep. Returns list of per-core input dicts."""
    import ml_dtypes
    bf16 = ml_dtypes.bfloat16

    collision_idxs = np.asarray(collision_idxs)
    vertices = np.asarray(vertices)
    faces = np.asarray(faces).astype(np.int64)
    joint_regressor = np.asarray(joint_regressor)

    # jr^T padded, f32 for the joints matmul; [S|M] rows in bf16 for the table
    jrt = np.zeros((NPAD, J), dtype=np.float32)
    jrt[:N, :] = joint_regressor.T.astype(np.float32)
    sm = np.zeros((NPAD, J2), dtype=bf16)
    sm[:N, 0:J] = jrt[:N].astype(bf16)
    sm[:N, J:J2] = (jrt[:N] != 0).astype(bf16)

    # per-(batch, face) table row: 3 x [S|M] = 144 bf16, padded to 256
    fsm_all = np.zeros((B, FPAD, E), dtype=bf16)
    fsm_all[:, :F, 0:3 * J2] = sm[faces.reshape(B, F * 3)].reshape(B, F, 3 * J2)

    vpad = np.zeros((B, NPAD, 3), dtype=np.float32)
    vpad[:, :N, :] = vertices.astype(np.float32)

    # gather index values: valid ? clip(cf) : F (zero row), + b*FPAD
    cidx = collision_idxs.astype(np.int32)
    valid = cidx[:, :, 0] >= 0
    sel = np.empty((2, B, C), dtype=np.int32)      # side 0 = recv, 1 = intr
    sel[0] = np.where(valid, np.maximum(cidx[:, :, 0], 0), F)
    sel[1] = np.where(valid, np.maximum(cidx[:, :, 1], 0), F)

    # c(q, t, a) = q*128 + t*8 + a; descriptor k = T*256 + side*128 + 16a + q
    cgrid = (np.arange(16)[:, None, None] * 128 +
             np.arange(16)[None, :, None] * 8 +
             np.arange(8)[None, None, :])          # [q, t, a]

    in_maps = []
    for core in range(NCORES):
        bs = slice(core * BPC, (core + 1) * BPC)
        v = np.empty((2 * 16, 2, 8, 16), dtype=np.int32)   # [T, side, a, q]
        for bb in range(BPC):
            for side in range(2):
                g = sel[side, core * BPC + bb][cgrid]      # [q, t, a]
                v[bb * 16:(bb + 1) * 16, side] = (
                    bb * FPAD + g.transpose(1, 2, 0))      # [t, a, q]
        wrapped = v.reshape(NIDX // 16, 16).T              # [q, slot]
        widx = np.tile(wrapped, (8, 1)).astype(np.int16)

        vc = np.zeros((NPAD, 6), dtype=np.float32)
        vc[:, 0:3] = vpad[core * BPC]
        vc[:, 3:6] = vpad[core * BPC + 1]

        msk = np.zeros((J2, J2), dtype=np.float32)
        msk[0:J, J:J2] = 1.0
        msk[J:J2, 0:J] = 1.0
        in_maps.append({
            "widx": widx,
            "jrt": np.ascontiguousarray(jrt.reshape(128, KCH * J)),
            "vc": np.ascontiguousarray(vc.reshape(128, KCH * 6)),
            "fsm": np.ascontiguousarray(
                fsm_all[bs].reshape(BPC * FPAD, E)),
            "msk": msk,
        })
    return in_maps


def kernel(collision_idxs, vertices, faces, joint_regressor):
    from concourse.bass_utils import run_bass_kernel_spmd

    nc = get_program()
    in_maps = make_in_maps(collision_idxs, vertices, faces, joint_regressor)
    res = run_bass_kernel_spmd(nc, in_maps, core_ids=list(range(NCORES)))
    num = 0.0
    den = 0.0
    for r in res.results:
        o = np.asarray(r["out"], dtype=np.float64).reshape(J2, 4)
        num += o[:, 0].sum() + o[:, 2].sum()
        den += o[J:J2, 1].sum() + o[J:J2, 3].sum()
    if den > 0:
        val = num / max(den, 1.0)
    else:
        val = 0.0
    return np.float32(val)
